# revision 22
# baseline (speedup 1.0000x reference)
"""Trainium2 Bass kernel v3: 4-head causal+ragged attention, one sample per core.

Key structure vs v2:
- Query chunks of 512; key blocks paired (2kb) per job.
- AV matmuls use fp8e4 DoubleRow (A.T@xA + B.T@xB): 2 key blocks per output
  column -> ~2x PE throughput on AV. exp output in fp8 with exp(s/4 - 2.5)
  shift so max value (~282) fits e4m3 (448).
- exp split: most jobs on ACT (fp8 out); a few off-diagonal jobs on DVE via
  one-op int16 Schraudolph (bf16-bits = i16(A*s + B)), consumed by bf16 AV.
- Causal diagonal masking via PE: accumulate -BIG upper-triangular constant
  into score PSUM (extra 128-col matmul), so exp -> 0 on both paths.
- Ragged masking via host-zeroed xkv (V+valid-col zero beyond len).
- Per-chunk finalize pipelined with later chunks; rb/py PSUM reuses the dead
  pctx chunk region (fits 8 PSUM banks: pctx 4 + st 2x2).
"""
import sys
sys.path.insert(0, '/opt/trn_rl_repo')
import numpy as np
import ml_dtypes
import concourse.bacc as bacc
import concourse.mybir as mybir
from concourse.tile import TileContext

F32 = mybir.dt.float32
BF16 = mybir.dt.bfloat16
FP8 = mybir.dt.float8e4
I16 = mybir.dt.int16
EXP = mybir.ActivationFunctionType.Exp
COPY = mybir.ActivationFunctionType.Copy
MULT = mybir.AluOpType.mult
ADD = mybir.AluOpType.add
DR = mybir.MatmulPerfMode.DoubleRow

S = 2048
D = 64
H = 4
DH = 16
CH = 512
NCH = S // CH
BIG = 1000.0
SHIFT = 3.0
SCH_A = 2.0 ** 23 / np.log(2.0) / 65536.0   # 184.6617 bf16-bits per e-unit
SCH_B = 16251.0                              # tuned for floor conversion
LAG = 8
FIN_LAG = 3


def build_nc(num_cores=8, loop_n=1, nkb=16, n_dve=22, stage="full", unroll=1):
    nkb_real = nkb
    nkb_even = min(16, 2 * ((nkb_real + 1) // 2))
    pairs = [(a, a + 1) for a in range(0, nkb_even, 2)]

    nc = bacc.Bacc("TRN2", target_bir_lowering=False, debug=False,
                   num_devices=num_cores)
    XBT = nc.dram_tensor("xbt", [D + 1, S], BF16, kind="ExternalInput").ap()
    XKV = nc.dram_tensor("xkv", [D + 1, S], BF16, kind="ExternalInput").ap()
    WALL = nc.dram_tensor("wall", [128, 960], BF16, kind="ExternalInput").ap()
    Y = nc.dram_tensor("y", [S, D], F32, kind="ExternalOutput").ap()

    # DVE (Schraudolph) jobs: pair 0 (kb 0,1) in chunks >= 1 is off-diagonal
    # (keys < 256 <= q), so no masks needed on that path.
    dve_jobs = set()
    cnt = 0
    for pi in (0, 1):
        for c in range(NCH - 1, 0, -1):
            for h in range(H):
                if cnt < n_dve:
                    dve_jobs.add((c, h, pi))
                    cnt += 1

    # per-chunk AV job count (for stop flags): jobs (h, pair) with 128a < end
    av_jobs_per_chunk = [
        H * sum(1 for (a, b) in pairs if 128 * a < CH * (c + 1))
        for c in range(NCH)
    ]

    import contextlib
    with TileContext(nc) as tc, nc.allow_low_precision(
            reason="bf16/fp8 matmuls, fp8 softmax weights, Schraudolph exp"):
        stack = contextlib.ExitStack()
        vp = stack.enter_context(tc.sbuf_pool(name="pers", bufs=1))
        v8 = [vp.tile([128, S], FP8, name=f"v8{h}") for h in range(H)]
        vB = [vp.tile([128, S], BF16, name=f"vB{h}") for h in range(H)]
        kH = [vp.tile([128, S], BF16, name=f"kH{h}") for h in range(H)]
        r4 = vp.tile([32, S], BF16, name="r4")
        nc.vector.memset(r4[:], 0.0)
        for h in range(H):
            nc.vector.memset(v8[h][:], 0.0)
            nc.vector.memset(vB[h][:], 0.0)
            nc.vector.memset(kH[h][:], 0.0)

        assert loop_n % unroll == 0 or loop_n == 1
        n_steps = loop_n // unroll if loop_n > 1 else 1
        n_body = unroll if loop_n > 1 else 1
        loop_cm = tc.For_i(0, n_steps, 1) if n_steps > 1 else contextlib.nullcontext()
        with stack, loop_cm, \
             tc.sbuf_pool(name="const", bufs=1) as cp, \
             tc.sbuf_pool(name="sb", bufs=1) as sp, \
             tc.sbuf_pool(name="ex8p", bufs=12) as ebp8, \
             tc.sbuf_pool(name="exdp", bufs=8) as ebpD, \
             tc.sbuf_pool(name="exbp", bufs=6) as ebpB, \
             tc.sbuf_pool(name="fin", bufs=2) as fp_, \
             tc.sbuf_pool(name="ys", bufs=2) as yp:
          for _body in range(n_body):
            wall = cp.tile([128, 960], BF16)
            nc.sync.dma_start(out=wall[:], in_=WALL[:])
            wq = wall[0:D + 1, 0:128]
            wk = wall[0:D + 1, 128:256]
            wv = wall[0:D + 1, 256:384]
            wp = wall[:, 384:448]
            eb4 = wall[:, 448:576]
            triu = wall[:, 576:704]
            iden = wall[:, 704:832]
            tril = wall[:, 832:960]

            xbt = sp.tile([D + 1, S], BF16)
            xkv = sp.tile([D + 1, S], BF16)
            for cc in range(2):
                cs = slice(cc * 1024, (cc + 1) * 1024)
                nc.sync.dma_start(out=xbt[:, cs], in_=XBT[:, cs])
                nc.gpsimd.dma_start(out=xkv[:, cs], in_=XKV[:, cs])

            qA = sp.tile([128, S], BF16)
            kS = sp.tile([128, S], BF16)

            # --- projections ---
            with tc.psum_pool(name="pg", bufs=3) as pg:
                for cc in range(4):
                    cl = slice(cc * 512, (cc + 1) * 512)
                    xs = xbt[0:D + 1, cl]
                    pq = pg.tile([128, 512], F32, tag="g")
                    nc.tensor.matmul(pq[:], wq, xs, start=True, stop=True)
                    pk = pg.tile([128, 512], F32, tag="g")
                    nc.tensor.matmul(pk[:], wk, xs, start=True, stop=True)
                    nc.scalar.activation(qA[:, cl], pq[:], COPY)
                    nc.vector.tensor_copy(kS[:, cl], pk[:])
                    for h in range(H):
                        hr = slice(32 * h, 32 * h + 32)
                        nc.gpsimd.tensor_copy(kH[h][hr, cl], kS[hr, cl])
                for g in range(4):
                    if 4 * g >= nkb_real:
                        break
                    pv = pg.tile([128, 512], F32, tag="g")
                    vi = 0
                    for i in range(4):
                        kb = 4 * g + i
                        if kb >= nkb_real:
                            break
                        nc.tensor.matmul(pv[:, i * 128:(i + 1) * 128],
                                         xkv[:, kb * 128:(kb + 1) * 128], wv,
                                         start=True, stop=True)
                        vi += 1
                    vS = sp.tile([128, 512], BF16, name=f"vS{g}")
                    nc.vector.tensor_copy(vS[:, 0:128 * vi], pv[:, 0:128 * vi])
                    srcv = vS.rearrange("p (k c) -> p k c", c=128)
                    for h in range(H):
                        hs = slice(4 + 32 * h, 4 + 32 * h + DH)
                        dst8 = v8[h][:, g * 512:(g + 1) * 512] \
                            .rearrange("p (k c) -> p k c", c=128)
                        nc.gpsimd.tensor_copy(dst8[:, 0:vi, hs], srcv[:, 0:vi, hs])
                        nc.gpsimd.tensor_copy(dst8[:, 0:vi, h:h + 1],
                                              srcv[:, 0:vi, 0:1])
                        dstB = vB[h][:, g * 512:(g + 1) * 512] \
                            .rearrange("p (k c) -> p k c", c=128)
                        nc.gpsimd.tensor_copy(dstB[:, 0:vi, hs],
                                              srcv[:, 0:vi, hs])
                        nc.gpsimd.tensor_copy(dstB[:, 0:vi, h:h + 1],
                                              srcv[:, 0:vi, 0:1])

            # --- attention ---
            def kslice(h, kb):
                return kH[h][:, kb * 128:(kb + 1) * 128]

            def qslice(h, c0, c1):
                return qA[:, c0:c1]

            with tc.psum_pool(name="pc", bufs=2) as pcq:
                pend = []
                chunk_started = [False] * NCH
                av_left = list(av_jobs_per_chunk)
                pctx_c = {}

                def finalize_head(c):
                    ccs = slice(CH * c, CH * (c + 1))
                    pc_ = pctx_c[c]
                    z0 = fp_.tile([128, CH], BF16, tag="z0")
                    nc.vector.tensor_copy(z0[:], pc_[:, 0:CH])
                    nc.vector.reciprocal(r4[0:H, ccs], pc_[0:H, 0:CH])
                    return z0

                def finalize_tail(c, z0):
                    ccs = slice(CH * c, CH * (c + 1))
                    pc_ = pctx_c[c]
                    nc.tensor.matmul(pc_[:, 0:CH], eb4[0:32, :], r4[:, ccs],
                                     start=True, stop=True,
                                     skip_group_check=True)
                    rbs = fp_.tile([128, CH], BF16, tag="rbs")
                    nc.vector.tensor_copy(rbs[:], pc_[:, 0:CH])
                    z = fp_.tile([128, CH], BF16, tag="z")
                    nc.gpsimd.tensor_mul(z[:], z0[:], rbs[:])
                    ys = yp.tile([128, 256], F32, tag="ys")
                    py = pc_[:, 0:256]
                    for tb in range(4):
                        nc.tensor.matmul(py[:, 64 * tb:64 * tb + 64],
                                         z[:, 128 * tb:128 * tb + 128], wp,
                                         start=True, stop=True,
                                         skip_group_check=True)
                    nc.vector.tensor_copy(ys[:], py)
                    ydst = Y[CH * c:CH * (c + 1), :] \
                        .rearrange("(g p) d -> p g d", p=128)
                    nc.sync.dma_start(out=ydst,
                                      in_=ys.rearrange("p (g d) -> p g d", d=D))

                def emit(job):
                    kind = job[0]
                    if kind == "fin":
                        finalize_tail(job[1], job[2])
                        return
                    if kind == "av8":
                        # off-diagonal: fp8 DoubleRow over kb pair, full width
                        _, c, h, a, t0, ex, stop_f = job
                        q0 = CH * c
                        lhs = v8[h][:, 128 * a:128 * a + 256] \
                            .rearrange("p (two f) -> p two f", two=2)
                        rhs = ex.rearrange("p (two f) -> p two f", two=2)
                        st_f = not chunk_started[c]
                        chunk_started[c] = True
                        nc.tensor.matmul(pctx_c[c][:, 0:CH], lhs, rhs,
                                         start=st_f, stop=stop_f,
                                         perf_mode=DR, skip_group_check=True)
                    else:  # avS: two solo bf16 matmuls, per-half trimmed
                        _, c, h, a, t0, bq0, b_run, ex, stop_f = job
                        q0 = CH * c
                        exb = ex[:].bitcast(BF16) if ex.dtype == I16 else ex[:]
                        st_f = not chunk_started[c]
                        chunk_started[c] = True
                        nc.tensor.matmul(pctx_c[c][:, t0:CH],
                                         vB[h][:, 128 * a:128 * a + 128],
                                         exb[:, t0:CH],
                                         start=st_f, stop=stop_f and not b_run,
                                         skip_group_check=True)
                        if b_run:
                            nc.tensor.matmul(pctx_c[c][:, bq0:CH],
                                             vB[h][:, 128 * a + 128:128 * a + 256],
                                             exb[:, CH + bq0:2 * CH],
                                             start=False, stop=stop_f,
                                             skip_group_check=True)

                with tc.psum_pool(name="ps", bufs=3) as ps:
                    for c in range(NCH):
                        q0 = CH * c
                        pctx_c[c] = pcq.tile([128, CH], F32, tag="pctx",
                                             name="pctxc")
                        for h in range(H):
                            for pi, (a, b) in enumerate(pairs):
                                if 128 * a >= CH * (c + 1):
                                    break
                                q0a = max(q0, 128 * a)
                                t0 = q0a - q0
                                a_diag = 128 * a >= q0
                                b_real = b < nkb_real
                                is_dve = (c, h, pi) in dve_jobs and not a_diag
                                st = ps.tile([128, 2 * CH], F32, tag="st")
                                stv = st.rearrange("p (two f) -> p two f", two=2)
                                # QK a (diag masking via Pool tril on ex)
                                nc.tensor.matmul(st[:, t0:CH], kslice(h, a),
                                                 qslice(h, q0 + t0, q0 + CH),
                                                 start=True, stop=True,
                                                 skip_group_check=True)
                                # QK b; on the diag path the sub-diagonal
                                # strip [t0, t0+128) is neither computed nor
                                # read (solo AV streams from bq0).
                                bq0 = t0 + 128 if a_diag else t0
                                b_diag = a_diag and b_real
                                b_run = b_real or not a_diag
                                if b_run:
                                    nc.tensor.matmul(st[:, CH + bq0:2 * CH],
                                                     kslice(h, b),
                                                     qslice(h, q0 + bq0, q0 + CH),
                                                     start=True, stop=True,
                                                     skip_group_check=True)
                                # exp
                                if is_dve:
                                    ex = ebpB.tile([128, 2 * CH], I16, tag="exB")
                                    nc.vector.tensor_scalar(
                                        ex[:], st[:], SCH_A * 0.25,
                                        SCH_B, MULT, ADD)
                                    job = ("avS", c, h, a, 0, 0, True, ex, None)
                                elif a_diag:
                                    ex = ebpD.tile([128, 2 * CH], BF16, tag="exD")
                                    exv = ex.rearrange("p (two f) -> p two f",
                                                       two=2)
                                    nc.scalar.activation(
                                        exv[:, 0:1, t0:CH], stv[:, 0:1, t0:CH],
                                        EXP, bias=0.0, scale=0.25)
                                    nc.gpsimd.tensor_mul(
                                        ex[:, t0:t0 + 128],
                                        ex[:, t0:t0 + 128], tril)
                                    if b_run:
                                        nc.scalar.activation(
                                            exv[:, 1:2, bq0:CH],
                                            stv[:, 1:2, bq0:CH],
                                            EXP, bias=0.0, scale=0.25)
                                        nc.gpsimd.tensor_mul(
                                            ex[:, CH + bq0:CH + bq0 + 128],
                                            ex[:, CH + bq0:CH + bq0 + 128],
                                            tril)
                                    job = ("avS", c, h, a, t0, bq0, b_run, ex,
                                           None)
                                else:
                                    ex = ebp8.tile([128, 2 * CH], FP8, tag="ex8")
                                    nc.scalar.activation(ex[:], st[:], EXP,
                                                         bias=0.0, scale=0.25)
                                    job = ("av8", c, h, a, t0, ex, None)
                                av_left[c] -= 1
                                stop_f = av_left[c] == 0
                                job = job[:-1] + (stop_f,)
                                pend.append(job)
                                if stop_f:
                                    pend.append(("finh", c))
                                while len(pend) > LAG:
                                    j = pend.pop(0)
                                    if j[0] == "finh":
                                        z0 = finalize_head(j[1])
                                        pend.append(("fin", j[1], z0))
                                    else:
                                        emit(j)
                    while pend:
                        j = pend.pop(0)
                        if j[0] == "finh":
                            z0 = finalize_head(j[1])
                            pend.append(("fin", j[1], z0))
                        else:
                            emit(j)
    nc.compile()
    return nc


def host_prep(x_b, len_b, W_qkv, W_proj, b_proj):
    bf = ml_dtypes.bfloat16
    xbt = np.zeros((D + 1, S), np.float32)
    xbt[0:D] = x_b.T
    xbt[D] = 1.0
    xkv = xbt.copy()
    xkv[:, len_b:] = 0.0
    wq = np.zeros((D + 1, 128), np.float32)
    wk = np.zeros((D + 1, 128), np.float32)
    wv = np.zeros((D + 1, 128), np.float32)
    wp = np.zeros((128, D), np.float32)
    eb4 = np.zeros((128, 128), np.float32)
    for h in range(H):
        wq[0:D, 32 * h:32 * h + DH] = W_qkv[DH * h:DH * h + DH, :].T
        wk[0:D, 32 * h:32 * h + DH] = W_qkv[D + DH * h:D + DH * h + DH, :].T
        wq[D, 32 * h + DH] = 4.0
        wk[D, 32 * h + DH] = -SHIFT
        wv[0:D, 4 + 32 * h:4 + 32 * h + DH] = \
            W_qkv[2 * D + DH * h:2 * D + DH * h + DH, :].T
        wv[D, h] = 1.0
        wp[4 + 32 * h:4 + 32 * h + DH, :] = W_proj[:, DH * h:DH * h + DH].T
        wp[h, :] = np.asarray(b_proj, np.float32) / H
        eb4[h, h] = 1.0
        eb4[h, 4 + 32 * h:4 + 32 * h + DH] = 1.0
    j = np.arange(128)
    triu = -BIG * (j[None, :] > j[:, None]).astype(np.float32)
    iden = np.eye(128, dtype=np.float32)
    wall = np.zeros((128, 960), np.float32)
    wall[0:D + 1, 0:128] = wq
    wall[0:D + 1, 128:256] = wk
    wall[0:D + 1, 256:384] = wv
    wall[:, 384:448] = wp
    wall[:, 448:576] = eb4
    wall[:, 576:704] = triu
    wall[:, 704:832] = iden
    wall[:, 832:960] = (j[:, None] <= j[None, :]).astype(np.float32)
    return {"xbt": xbt.astype(bf), "xkv": xkv.astype(bf),
            "wall": wall.astype(bf)}


_RUNNER = None


def _build_runner(nc, n_cores=8):
    import jax
    from jax.sharding import Mesh, PartitionSpec
    from jax.experimental.shard_map import shard_map
    from concourse.bass2jax import (_bass_exec_p, install_neuronx_cc_hook,
                                    partition_id_tensor)
    install_neuronx_cc_hook()
    partition_name = nc.partition_id_tensor.name if nc.partition_id_tensor else None
    in_names, out_names, out_avals, zero_outs = [], [], [], []
    for alloc in nc.m.functions[0].allocations:
        if not isinstance(alloc, mybir.MemoryLocationSet):
            continue
        name = alloc.memorylocations[0].name
        if alloc.kind == "ExternalInput":
            if name != partition_name:
                in_names.append(name)
        elif alloc.kind == "ExternalOutput":
            shape = tuple(alloc.tensor_shape)
            dtype = mybir.dt.np(alloc.dtype)
            out_names.append(name)
            out_avals.append(jax.core.ShapedArray(shape, dtype))
            zero_outs.append(np.zeros(shape, dtype))
    n_params = len(in_names)
    n_outs = len(out_avals)
    all_in_names = list(in_names) + list(out_names)
    if partition_name is not None:
        all_in_names.append(partition_name)
    donate = tuple(range(n_params, n_params + n_outs))

    def _body(*args):
        operands = list(args)
        if partition_name is not None:
            operands.append(partition_id_tensor())
        outs = _bass_exec_p.bind(
            *operands,
            out_avals=tuple(out_avals),
            in_names=tuple(all_in_names),
            out_names=tuple(out_names),
            lowering_input_output_aliases=(),
            sim_require_finite=True,
            sim_require_nnan=True,
            nc=nc,
        )
        return tuple(outs)

    devices = jax.devices()[:n_cores]
    mesh = Mesh(np.asarray(devices), ("core",))
    in_specs = (PartitionSpec("core"),) * (n_params + n_outs)
    out_specs = (PartitionSpec("core"),) * n_outs
    sharded = jax.jit(
        shard_map(_body, mesh=mesh, in_specs=in_specs, out_specs=out_specs,
                  check_rep=False),
        donate_argnums=donate, keep_unused=True)

    def run(in_maps):
        import jax
        per_core = [[np.asarray(m[n]) for n in in_names] for m in in_maps]
        concat_in = [np.concatenate([per_core[c][i] for c in range(n_cores)], axis=0)
                     for i in range(n_params)]
        concat_zeros = [np.zeros((n_cores * z.shape[0], *z.shape[1:]), z.dtype)
                        for z in zero_outs]
        out_arrs = sharded(*concat_in, *concat_zeros)
        jax.block_until_ready(out_arrs)
        return [
            {name: np.asarray(out_arrs[i]).reshape(n_cores, *out_avals[i].shape)[c]
             for i, name in enumerate(out_names)}
            for c in range(n_cores)
        ]
    return run


def _numpy_fallback(x, attn_mask, W_qkv, W_proj, b_proj):
    B, S_, D_ = x.shape
    qkv = x @ W_qkv.T
    qkv = qkv.reshape(B, S_, 3, H, DH).transpose(2, 0, 3, 1, 4)
    q, k, v = qkv[0], qkv[1], qkv[2]
    s = np.einsum('bhqd,bhkd->bhqk', q, k).astype(np.float32) / np.sqrt(DH)
    neg = np.finfo(np.float32).min
    s = np.where(attn_mask, s, neg)
    s = s - s.max(-1, keepdims=True)
    p = np.exp(s)
    p = p / p.sum(-1, keepdims=True)
    ctx = np.einsum('bhqk,bhkd->bhqd', p, v)
    ctx = ctx.transpose(0, 2, 1, 3).reshape(B, S_, D_)
    return (ctx @ W_proj.T + b_proj).astype(np.float32)


def kernel(x, attn_mask, W_qkv, W_proj, b_proj):
    global _RUNNER
    x = np.asarray(x, np.float32)
    attn_mask = np.asarray(attn_mask)
    W_qkv = np.asarray(W_qkv, np.float32)
    W_proj = np.asarray(W_proj, np.float32)
    b_proj = np.asarray(b_proj, np.float32)
    B = x.shape[0]
    m = attn_mask[:, 0]
    lens = m[:, -1, :].sum(-1).astype(np.int64)
    pos = np.arange(S)
    causal = pos[:, None] >= pos[None, :]
    structured = bool((lens >= 1).all()) and all(
        np.array_equal(m[b], causal & (pos[None, :] < lens[b])) for b in range(B))
    if not (structured and B == 8 and x.shape == (8, S, D)):
        return _numpy_fallback(x, attn_mask, W_qkv, W_proj, b_proj)
    nkb = int(-(-int(lens.max()) // 128))
    if _RUNNER is None or _RUNNER[0] != nkb:
        nc = build_nc(num_cores=8, nkb=nkb, n_dve=22)
        _RUNNER = (nkb, _build_runner(nc, 8))
    in_maps = [host_prep(x[b], int(lens[b]), W_qkv, W_proj, b_proj)
               for b in range(B)]
    results = _RUNNER[1](in_maps)
    return np.stack([results[c]["y"] for c in range(8)]).astype(np.float32)
